# revision 26
# baseline (speedup 1.0000x reference)
"""Multi-head attention (B=2, S=2048, H=16, D=64) on 8 TRN2 NeuronCores.

Sharding: data parallel on batch (2) x tensor parallel on heads (16 -> 4 per
core).  Core c handles batch c//4 and heads [4*(c%4), 4*(c%4)+4).  Each core
projects q/k/v for its head group from its batch's activations, runs the
full S x S attention for its 4 heads, and writes ctx in [head, D, S] layout.
The host transposes/concatenates shards (not part of HW exec time).

Device kernel (per core, identical SPMD program, no collectives):
  - qT/kT computed directly in [D, S] layout (head pairs packed into 128
    partitions) so the scores matmul needs no transposes.
  - scoresT tiles [S_k=128, S_q] = kT_chunk.T @ qTz; softmax denominator via
    ones columns appended to v (one matmul stream produces ctx and denom).
  - exp on the scalar engine with the 1/sqrt(D) scale folded in; bf16
    matmul operands, f32 PSUM accumulation; final normalize = DVE divide.
  - padding mask folded into v_aug row zeroing (exp(x-1e4) underflows to 0
    in f32, so zeroing masked key rows is exactly equivalent).

Scheduling (see the block schedule below):
  - 8 blocks B(p,qc); block 1 carries only its mandatory projections
    (kT0 all keys, qT0 qc0, all v chunks, plus Q01 prefetch); the
    remaining 10 projection groups are spread one or two per block so the
    scalar-engine-bound pair-1 phase keeps the PE busy too.
  - ctx matmuls lag one (block, kc2) step behind the score matmuls
    (software pipelining across block boundaries) so the PE computes the
    next scores while the scalar engine is on the previous exps.
  - DMA: host packs every tensor per-partition-contiguous so each piece is
    one dma_start with 128 large descriptors.  The DMA fabric round-robins
    all in-flight transfers at ~300GB/s aggregate, so arrival priority =
    not being in flight with anything else: later pieces are gated on
    earlier completions via tiny gpsimd reads (cascade), and only the
    block-1-critical pieces (x qc0, pair-0 weights) issue immediately.
  - memset-fed warmup matmuls bridge the DMA head so the PE never idles
    long enough for the clock governor to hold it at 1.2GHz.
  - epilogue per head: copy denominator psum->sbuf, fast reciprocal,
    multiply straight out of psum (no [128,512] staging copy).
"""

import numpy as np
import ml_dtypes

import concourse.bass as bass
import concourse.tile as tile
from concourse import bacc, mybir
from concourse.bass_utils import run_bass_kernel_spmd

B, S, H, D = 2, 2048, 16, 64
HID = H * D
NCORES = 8
HPC = 4               # heads per core
COLS = HPC * D        # 256 projection columns per core
KC = HID // 128       # 8 contraction chunks for projections
QC = S // 512         # 4 query chunks of 512
MC = S // 128         # 16 key chunks of 128

BF16 = mybir.dt.bfloat16
F32 = mybir.dt.float32
np_bf16 = ml_dtypes.bfloat16

N_WARM = 8            # memset-fed filler matmuls bridging the DMA head

_CACHE = {}


def build(apply_mask: bool) -> bass.Bass:
    nc = bacc.Bacc(None, target_bir_lowering=False, debug=False)

    # Host-packed, per-partition-contiguous layouts (one big descriptor per
    # partition per piece): xP[qc][p][kc][c], w{q,k}P[pair][p][kc][c],
    # wvP[p][kc][c].
    xP = nc.declare_dram_parameter("xP", [QC, 128, KC, 512], BF16, isOutput=False)
    wqP = nc.declare_dram_parameter("wqP", [2, 128, KC, 128], BF16, isOutput=False)
    wkP = nc.declare_dram_parameter("wkP", [2, 128, KC, 128], BF16, isOutput=False)
    wvP = nc.declare_dram_parameter("wvP", [128, KC, COLS], BF16, isOutput=False)
    bq = nc.declare_dram_parameter("bq", [128, 2], F32, isOutput=False)
    bk = nc.declare_dram_parameter("bk", [128, 2], F32, isOutput=False)
    bv = nc.declare_dram_parameter("bv", [128, COLS], F32, isOutput=False)
    if apply_mask:
        mm_in = nc.declare_dram_parameter("maskm", [128, MC], F32, isOutput=False)
    out_ext = nc.declare_dram_parameter("out", [HPC, D, S], F32, isOutput=True)

    with tile.TileContext(nc) as tc:
        with (
            tc.tile_pool(name="singles", bufs=1) as singles,
            tc.tile_pool(name="work", bufs=4) as work,
            tc.tile_pool(name="psum", bufs=2, space="PSUM") as psum,
        ):
            wq_sb = singles.tile([128, 2, KC, 128], BF16)
            wk_sb = singles.tile([128, 2, KC, 128], BF16)
            wv_sb = singles.tile([128, KC, COLS], BF16)
            x_sb = singles.tile([128, QC, KC, 512], BF16)
            bq_sb = singles.tile([128, 2], F32)
            bk_sb = singles.tile([128, 2], F32)
            bv_sb = singles.tile([128, COLS], F32)
            if apply_mask:
                mm_sb = singles.tile([128, MC], F32)
            zt = singles.tile([128, 512], BF16)

            # ---- input DMA ----
            # Immediate wave: only what block 1 needs first (x qc0 + pair-0
            # weights + biases); ~1.5MB in flight -> all landed by ~14us.
            nc.sync.dma_start(out=x_sb[:, 0], in_=xP[0][:, :, :])
            nc.scalar.dma_start(out=wk_sb[:, 0], in_=wkP[0][:, :, :])
            nc.scalar.dma_start(out=wq_sb[:, 0], in_=wqP[0][:, :, :])
            nc.gpsimd.dma_start(out=bk_sb, in_=bk[:, :])
            nc.gpsimd.dma_start(out=bq_sb, in_=bq[:, :])
            if apply_mask:
                nc.gpsimd.dma_start(out=mm_sb, in_=mm_in[:, :])

            # Cascade: each later piece may only enter the shared DMA pipe
            # after an earlier piece has fully landed.  The engine
            # scheduler reorders by data deps (program order is not
            # preserved), so the gate is a tiny gpsimd copy READING the
            # trigger tile and WRITING into the gated DMA's destination —
            # the write-after-write dependency forces the DMA to wait.
            def gate(trig_qc, dst_ap):
                nc.gpsimd.tensor_copy(out=dst_ap,
                                      in_=x_sb[0:1, trig_qc, 7, 508:512])

            gate(0, wv_sb[0:1, 0, 0:4])
            nc.gpsimd.dma_start(out=wv_sb, in_=wvP[:, :, :])
            gate(0, bv_sb[0:1, 0:4])
            nc.gpsimd.dma_start(out=bv_sb, in_=bv[:, :])
            gate(0, x_sb[0:1, 1, 0, 0:4])
            nc.gpsimd.dma_start(out=x_sb[:, 1], in_=xP[1][:, :, :])
            gate(1, x_sb[0:1, 2, 0, 0:4])
            nc.gpsimd.dma_start(out=x_sb[:, 2], in_=xP[2][:, :, :])
            gate(1, wk_sb[0:1, 1, 0, 0:4])
            nc.gpsimd.dma_start(out=wk_sb[:, 1], in_=wkP[1][:, :, :])
            gate(2, x_sb[0:1, 3, 0, 0:4])
            nc.gpsimd.dma_start(out=x_sb[:, 3], in_=xP[3][:, :, :])
            gate(2, wq_sb[0:1, 1, 0, 0:4])
            nc.gpsimd.dma_start(out=wq_sb[:, 1], in_=wqP[1][:, :, :])

            # ---- static SBUF prep ----
            # All on GpSimd: the DVE must stay free for the K00/Q00 bias
            # adds that gate the first scores (a memset queued ahead of
            # them on the DVE stalls the whole pipeline start).
            nc.gpsimd.memset(zt, 0.0)
            # qTz zero halves are written once for the full S up front
            # (variant 0: head-b rows zero; variant 1: head-a rows zero)
            qTz = singles.tile([128, 2, 2, S], BF16)
            kT = singles.tile([128, 2, S], BF16)
            nc.gpsimd.memset(qTz[64:128, 0, 0, :], 0.0)
            nc.gpsimd.memset(qTz[0:64, 0, 1, :], 0.0)
            # v_aug: [128, key_chunk, head, 128]; cols 64:128 are ones, so
            # the ctx matmul emits the softmax denominator into psum
            # partitions 64:128 at no extra cost (matmul cost is N-bound)
            v_aug = singles.tile([128, MC, HPC, 128], BF16)
            nc.gpsimd.memset(v_aug[:, :, :, 64:128], 1.0)
            nc.gpsimd.memset(qTz[64:128, 1, 0, :], 0.0)
            nc.gpsimd.memset(qTz[0:64, 1, 1, :], 0.0)

            # ---- projection helpers ----
            def project_T_qc(dst, w_sb, b_sb, p, qc, zpad=False):
                ps = psum.tile([128, 512], F32, tag="proj_ps", name=f"pt_{nc.next_id()}")
                for kc in range(KC):
                    nc.tensor.matmul(
                        ps,
                        lhsT=w_sb[:, p, kc, :],
                        rhs=x_sb[:, qc, kc, :],
                        start=(kc == 0), stop=(kc == KC - 1),
                    )
                qsl = slice(qc * 512, (qc + 1) * 512)
                if zpad:
                    nc.vector.tensor_tensor(
                        out=dst[0:64, p, 0, qsl],
                        in0=ps[0:64, :],
                        in1=b_sb[0:64, p:p + 1].to_broadcast([64, 512]),
                        op=mybir.AluOpType.add,
                    )
                    nc.vector.tensor_tensor(
                        out=dst[64:128, p, 1, qsl],
                        in0=ps[64:128, :],
                        in1=b_sb[64:128, p:p + 1].to_broadcast([64, 512]),
                        op=mybir.AluOpType.add,
                    )
                else:
                    nc.vector.tensor_tensor(
                        out=dst[:, p, qsl],
                        in0=ps,
                        in1=b_sb[:, p:p + 1].to_broadcast([128, 512]),
                        op=mybir.AluOpType.add,
                    )

            def project_v_chunk(mc):
                ps = psum.tile([128, COLS], F32, tag="proj_ps", name=f"pv_{nc.next_id()}")
                for kc in range(KC):
                    nc.tensor.matmul(
                        ps,
                        lhsT=x_sb[:, mc // 4, kc, (mc % 4) * 128:(mc % 4 + 1) * 128],
                        rhs=wv_sb[:, kc, :],
                        start=(kc == 0), stop=(kc == KC - 1),
                    )
                nc.vector.tensor_tensor(
                    out=v_aug[:, mc, :, 0:64],
                    in0=ps[:, :].rearrange("p (h d) -> p h d", h=HPC),
                    in1=bv_sb.rearrange("p (h d) -> p h d", h=HPC),
                    op=mybir.AluOpType.add,
                )
                if apply_mask:
                    nc.vector.tensor_tensor(
                        out=v_aug[:, mc, :, :],
                        in0=v_aug[:, mc, :, :],
                        in1=mm_sb[:, mc:mc + 1, None].to_broadcast([128, HPC, 128]),
                        op=mybir.AluOpType.mult,
                    )

            # Named projection groups: K{p}{j} = kT pair p keys j*512..,
            # Q{p}{j} = qTz pair p queries j*512.. (zero-padded variants).
            def K(p, j):
                return lambda: project_T_qc(kT, wk_sb, bk_sb, p, j)

            def Q(p, j):
                return lambda: project_T_qc(qTz, wq_sb, bq_sb, p, j, zpad=True)

            def V2(mc):
                return lambda: (project_v_chunk(mc), project_v_chunk(mc + 1))

            def emit_ctx(pend):
                p, qc, kc2, e_a, e_b, ctx_a, ctx_b = pend
                ha, hb = 2 * p, 2 * p + 1
                for i, kc in enumerate((2 * kc2, 2 * kc2 + 1)):
                    csl = slice(i * 512, (i + 1) * 512)
                    nc.tensor.matmul(
                        ctx_a, lhsT=v_aug[:, kc, ha, :], rhs=e_a[:, csl],
                        start=(kc == 0), stop=(kc == MC - 1))
                    nc.tensor.matmul(
                        ctx_b, lhsT=v_aug[:, kc, hb, :], rhs=e_b[:, csl],
                        start=(kc == 0), stop=(kc == MC - 1))

            def emit_epilogue(p, qc, ctx_a, ctx_b, last=False):
                # denominator (psum rows 64:128) -> sbuf base-0 tile for the
                # custom-DVE reciprocal; multiply reads ctx psum directly so
                # there is no staging copy of the whole tile.  In the last
                # block the out-DMA issues split across the SP and Scalar
                # sequencers to shorten the tail.
                for h, ctx in ((2 * p, ctx_a), (2 * p + 1, ctx_b)):
                    d0_sb = work.tile([64, 512], F32, tag="den0", name=f"d0_{nc.next_id()}")
                    nc.vector.tensor_copy(out=d0_sb, in_=ctx[64:128, :])
                    d_sb = work.tile([64, 512], F32, tag="den", name=f"d_{nc.next_id()}")
                    nc.vector.reciprocal_approx_fast(out=d_sb, in_=d0_sb)
                    o_sb = work.tile([64, 512], F32, tag="outt", name=f"o_{nc.next_id()}")
                    nc.vector.tensor_tensor(
                        out=o_sb, in0=ctx[0:64, :],
                        in1=d_sb,
                        op=mybir.AluOpType.mult)
                    for piece in range(2):
                        eng = (nc.scalar if piece else nc.sync) if last else nc.sync
                        eng.dma_start(
                            out=out_ext[h][:, qc * 512 + piece * 256:
                                           qc * 512 + (piece + 1) * 256],
                            in_=o_sb[:, piece * 256:(piece + 1) * 256])

            # ---- block schedule ----
            # B1 must own kT0 (all keys), qT0 qc0, every v chunk, and the
            # Q01 prefetch; the other 10 groups go one or two per block so
            # blocks 6-8 (scalar-bound: 16 exps = 16.6us vs 13.7us of
            # attention matmuls) still keep the PE fed.
            blocks = [(0, 0), (0, 1), (0, 2), (0, 3),
                      (1, 0), (1, 1), (1, 2), (1, 3)]
            hooks = [
                # V pair 2j is consumed by ctx(j), which the software
                # pipeline emits during iteration j+1 — so V pairs sit in
                # hook j+1, AFTER that iteration's scores, keeping block
                # 1's first scores/exps ahead of any wv-DMA stall.
                {1: lambda: (K(0, 1)(), V2(0)()),
                 2: V2(2), 3: lambda: (K(0, 2)(), V2(4)()),
                 4: V2(6), 5: lambda: (K(0, 3)(), V2(8)()),
                 6: V2(10), 7: lambda: (Q(0, 1)(), V2(12)(), V2(14)())},
                {1: Q(0, 2), 5: K(1, 0)},
                {1: Q(0, 3), 5: K(1, 1)},
                {1: Q(1, 0), 5: K(1, 2)},
                {1: Q(1, 1), 5: K(1, 3)},
                {1: Q(1, 2)},
                {1: Q(1, 3)},
                {},
            ]

            # ---- warmup + first projections ----
            warm_ps = psum.tile([128, 512], F32, tag="proj_ps", name="warm_ps")
            k00_ps = psum.tile([128, 512], F32, tag="proj_ps", name="k00_ps")

            def warm(n, cols=512):
                # singleton-group filler matmuls (start+stop, no accumulate)
                for _ in range(n):
                    nc.tensor.matmul(warm_ps[:, 0:cols], lhsT=zt[:, 0:128],
                                     rhs=zt[:, 0:cols], start=True, stop=True)

            warm(N_WARM)
            # K00 may still be paced by the x qc0 / wk arrivals; short
            # N=128 fillers between steps keep the PE busy so the clock
            # ramp (needs ~3us continuous) isn't reset by the waits.
            for kc in range(KC):
                nc.tensor.matmul(
                    k00_ps, lhsT=wk_sb[:, 0, kc, :],
                    rhs=x_sb[:, 0, kc, :],
                    start=(kc == 0), stop=(kc == KC - 1))
                if kc < KC - 1:
                    warm(3, cols=128)
            nc.vector.tensor_tensor(
                out=kT[:, 0, 0:512], in0=k00_ps,
                in1=bk_sb[:, 0:1].to_broadcast([128, 512]),
                op=mybir.AluOpType.add)
            project_T_qc(qTz, wq_sb, bq_sb, 0, 0, True)   # Q00

            # ---- software-pipelined attention ----
            pend = None
            for bi, (p, qc) in enumerate(blocks):
                qsl = slice(qc * 512, (qc + 1) * 512)
                ctx_a = psum.tile([128, 512], F32, tag="ctx", name=f"ca_{nc.next_id()}")
                ctx_b = psum.tile([128, 512], F32, tag="ctx", name=f"cb_{nc.next_id()}")
                for kc2 in range(MC // 2):
                    kc0, kc1 = 2 * kc2, 2 * kc2 + 1
                    s_a = psum.tile([128, 1024], F32, tag="sps", name=f"sa_{nc.next_id()}")
                    s_b = psum.tile([128, 1024], F32, tag="sps", name=f"sb_{nc.next_id()}")
                    # paired row-group matmuls: head a on array rows 0:63,
                    # head b on rows 64:127 run concurrently
                    for i, kc in enumerate((kc0, kc1)):
                        ksl = slice(kc * 128, (kc + 1) * 128)
                        csl = slice(i * 512, (i + 1) * 512)
                        nc.tensor.matmul(
                            s_a[:, csl], lhsT=kT[:, p, ksl], rhs=qTz[:, p, 0, qsl],
                            start=True, stop=True)
                        nc.tensor.matmul(
                            s_b[:, csl], lhsT=kT[:, p, ksl], rhs=qTz[:, p, 1, qsl],
                            start=True, stop=True)
                    hook = hooks[bi].get(kc2)
                    if hook is not None:
                        hook()
                    e_a = work.tile([128, 1024], BF16, tag="expT", name=f"ea_{nc.next_id()}")
                    e_b = work.tile([128, 1024], BF16, tag="expT", name=f"eb_{nc.next_id()}")
                    nc.scalar.activation(e_a, s_a, mybir.ActivationFunctionType.Exp,
                                         scale=0.125)
                    nc.scalar.activation(e_b, s_b, mybir.ActivationFunctionType.Exp,
                                         scale=0.125)
                    if pend is not None:
                        emit_ctx(pend)
                        if pend[2] == MC // 2 - 1:
                            emit_epilogue(pend[0], pend[1], pend[5], pend[6])
                    pend = (p, qc, kc2, e_a, e_b, ctx_a, ctx_b)
            emit_ctx(pend)
            emit_epilogue(pend[0], pend[1], pend[5], pend[6], last=True)

    nc.compile()
    return nc


def _get_nc(apply_mask: bool) -> bass.Bass:
    if apply_mask not in _CACHE:
        _CACHE[apply_mask] = build(apply_mask)
    return _CACHE[apply_mask]


def _in_maps(x, mask, Wq, bq, Wk, bk, Wv, bv, apply_mask):
    # per-partition-contiguous packing (see build()):
    #   xP[qc, p, kc, c]  = x[b][qc*512+c, kc*128+p]
    #   wP[pair, p, kc, c] = W[kc*128+p, pair*128+c]   (per-core col slice)
    #   wvP[p, kc, c]      = Wv[kc*128+p, c]
    xP_b = [np.ascontiguousarray(
        x[b].reshape(QC, 512, KC, 128).transpose(0, 3, 2, 1)).astype(np_bf16)
        for b in range(B)]

    def pack_w(W, cs):
        Wc = np.asarray(W[:, cs])  # [HID, COLS]
        return np.ascontiguousarray(
            Wc.reshape(KC, 128, 2, 128).transpose(2, 1, 0, 3)).astype(np_bf16)

    maps = []
    for c in range(NCORES):
        b, hg = c // 4, c % 4
        cs = slice(hg * COLS, (hg + 1) * COLS)
        m = {
            "xP": xP_b[b],
            "wqP": pack_w(Wq, cs),
            "wkP": pack_w(Wk, cs),
            "wvP": np.ascontiguousarray(
                Wv[:, cs].reshape(KC, 128, COLS).transpose(1, 0, 2)).astype(np_bf16),
            "bq": np.ascontiguousarray(bq[cs].reshape(2, 128).T).astype(np.float32),
            "bk": np.ascontiguousarray(bk[cs].reshape(2, 128).T).astype(np.float32),
            "bv": np.ascontiguousarray(
                np.broadcast_to(bv[cs], (128, COLS))).astype(np.float32),
        }
        if apply_mask:
            m["maskm"] = np.ascontiguousarray(
                mask[b].astype(np.float32).reshape(MC, 128).T)
        maps.append(m)
    return maps


def _ensure_ntff_hook():
    """The agent image's antenv lacks axon_hooks; synthesize it so
    run_bass_kernel_spmd(trace=True) can reach the axon NTFF profiler."""
    import sys as _sys
    import types as _types
    try:
        from antenv import axon_hooks  # noqa: F401
        return
    except ImportError:
        pass
    import antenv
    mod = _types.ModuleType("antenv.axon_hooks")
    _hook = [None]
    mod.set_axon_ntff_profile_hook = lambda h: _hook.__setitem__(0, h)
    mod.get_axon_ntff_profile_hook = lambda: _hook[0]
    _sys.modules["antenv.axon_hooks"] = mod
    antenv.axon_hooks = mod
    from trn_agent_boot.trn_boot import _ntff_profile_via_ctypes
    mod.set_axon_ntff_profile_hook(
        _ntff_profile_via_ctypes("/opt/axon/libaxon_pjrt.so"))


def run(inputs: dict, trace: bool = False):
    if trace:
        _ensure_ntff_hook()
    x = np.asarray(inputs["x"], dtype=np.float32)
    mask = np.asarray(inputs["mask"])
    apply_mask = not bool((mask == 1).all())
    nc = _get_nc(apply_mask)
    maps = _in_maps(x, mask, np.asarray(inputs["Wq"], np.float32),
                    np.asarray(inputs["bq"], np.float32),
                    np.asarray(inputs["Wk"], np.float32),
                    np.asarray(inputs["bk"], np.float32),
                    np.asarray(inputs["Wv"], np.float32),
                    np.asarray(inputs["bv"], np.float32), apply_mask)
    res = run_bass_kernel_spmd(nc, maps, core_ids=list(range(NCORES)), trace=trace)
    out = np.empty((B, S, HID), dtype=np.float32)
    for c in range(NCORES):
        b, hg = c // 4, c % 4
        cs = slice(hg * COLS, (hg + 1) * COLS)
        ctxT = res.results[c]["out"]          # [HPC, D, S]
        out[b, :, cs] = ctxT.transpose(2, 0, 1).reshape(S, COLS)
    return out, res


def kernel(**inputs) -> np.ndarray:
    out, _ = run(inputs)
    return out


# revision 28
# speedup vs baseline: 1.2295x; 1.2295x over previous
"""Multi-head attention (B=2, S=2048, H=16, D=64) on 8 TRN2 NeuronCores.

Sharding: data parallel on batch (2) x tensor parallel on heads (16 -> 4 per
core).  Core c handles batch c//4 and heads [4*(c%4), 4*(c%4)+4).  Each core
projects q/k/v for its head group from its batch's activations, runs the
full S x S attention for its 4 heads, and writes ctx in [head, D, S] layout.
The host transposes/concatenates shards (not part of HW exec time).

Device kernel (per core, identical SPMD program, no collectives):
  - qT/kT computed directly in [D, S] layout (head pairs packed into 128
    partitions) so the scores matmul needs no transposes.
  - scoresT tiles [S_k=128, S_q] = kT_chunk.T @ qTz; softmax denominator via
    ones columns appended to v (one matmul stream produces ctx and denom).
  - exp on the scalar engine with the 1/sqrt(D) scale folded in; bf16
    matmul operands, f32 PSUM accumulation; final normalize = DVE divide.
  - padding mask folded into v_aug row zeroing (exp(x-1e4) underflows to 0
    in f32, so zeroing masked key rows is exactly equivalent).

Scheduling (see the block schedule below):
  - 8 blocks B(p,qc); block 1 carries only its mandatory projections
    (kT0 all keys, qT0 qc0, all v chunks, plus Q01 prefetch); the
    remaining 10 projection groups are spread one or two per block so the
    scalar-engine-bound pair-1 phase keeps the PE busy too.
  - ctx matmuls lag one (block, kc2) step behind the score matmuls
    (software pipelining across block boundaries) so the PE computes the
    next scores while the scalar engine is on the previous exps.
  - DMA: host packs every tensor per-partition-contiguous so each piece is
    one dma_start with 128 large descriptors.  The DMA fabric round-robins
    all in-flight transfers at ~300GB/s aggregate, so arrival priority =
    not being in flight with anything else: later pieces are gated on
    earlier completions via tiny gpsimd reads (cascade), and only the
    block-1-critical pieces (x qc0, pair-0 weights) issue immediately.
  - memset-fed warmup matmuls bridge the DMA head so the PE never idles
    long enough for the clock governor to hold it at 1.2GHz.
  - epilogue per head: copy denominator psum->sbuf, fast reciprocal,
    multiply straight out of psum (no [128,512] staging copy).
"""

import numpy as np
import ml_dtypes

import concourse.bass as bass
import concourse.tile as tile
from concourse import bacc, mybir
from concourse.bass_utils import run_bass_kernel_spmd

B, S, H, D = 2, 2048, 16, 64
HID = H * D
NCORES = 8
HPC = 4               # heads per core
COLS = HPC * D        # 256 projection columns per core
KC = HID // 128       # 8 contraction chunks for projections
QC = S // 512         # 4 query chunks of 512
MC = S // 128         # 16 key chunks of 128

BF16 = mybir.dt.bfloat16
F32 = mybir.dt.float32
np_bf16 = ml_dtypes.bfloat16

N_WARM = 8            # memset-fed filler matmuls bridging the DMA head

_CACHE = {}


def build(apply_mask: bool) -> bass.Bass:
    nc = bacc.Bacc(None, target_bir_lowering=False, debug=False)

    # Host-packed, per-partition-contiguous layouts (one big descriptor per
    # partition per piece): xP[qc][p][kc][c], w{q,k}P[pair][p][kc][c],
    # wvP[p][kc][c].
    xP = nc.declare_dram_parameter("xP", [QC, 128, KC, 512], BF16, isOutput=False)
    wqP = nc.declare_dram_parameter("wqP", [2, 128, KC, 128], BF16, isOutput=False)
    wkP = nc.declare_dram_parameter("wkP", [2, 128, KC, 128], BF16, isOutput=False)
    wvP = nc.declare_dram_parameter("wvP", [128, KC, COLS], BF16, isOutput=False)
    bq = nc.declare_dram_parameter("bq", [128, 2], F32, isOutput=False)
    bk = nc.declare_dram_parameter("bk", [128, 2], F32, isOutput=False)
    bv = nc.declare_dram_parameter("bv", [128, COLS], F32, isOutput=False)
    if apply_mask:
        mm_in = nc.declare_dram_parameter("maskm", [128, MC], F32, isOutput=False)
    out_ext = nc.declare_dram_parameter("out", [HPC, D, S], F32, isOutput=True)

    with tile.TileContext(nc) as tc:
        with (
            tc.tile_pool(name="singles", bufs=1) as singles,
            tc.tile_pool(name="work", bufs=4) as work,
            tc.tile_pool(name="psum", bufs=2, space="PSUM") as psum,
        ):
            wq_sb = singles.tile([128, 2, KC, 128], BF16)
            wk_sb = singles.tile([128, 2, KC, 128], BF16)
            wv_sb = singles.tile([128, KC, COLS], BF16)
            x_sb = singles.tile([128, QC, KC, 512], BF16)
            bq_sb = singles.tile([128, 2], F32)
            bk_sb = singles.tile([128, 2], F32)
            bv_sb = singles.tile([128, COLS], F32)
            if apply_mask:
                mm_sb = singles.tile([128, MC], F32)
            zt = singles.tile([128, 512], BF16)

            # ---- input DMA ----
            # Immediate wave: only what block 1 needs first (x qc0 + pair-0
            # weights + biases); ~1.5MB in flight -> all landed by ~14us.
            nc.sync.dma_start(out=x_sb[:, 0], in_=xP[0][:, :, :])
            nc.scalar.dma_start(out=wk_sb[:, 0], in_=wkP[0][:, :, :])
            nc.scalar.dma_start(out=wq_sb[:, 0], in_=wqP[0][:, :, :])
            nc.gpsimd.dma_start(out=bk_sb, in_=bk[:, :])
            nc.gpsimd.dma_start(out=bq_sb, in_=bq[:, :])
            if apply_mask:
                nc.gpsimd.dma_start(out=mm_sb, in_=mm_in[:, :])

            # Cascade: each later piece may only enter the shared DMA pipe
            # after an earlier piece has fully landed.  The engine
            # scheduler reorders by data deps (program order is not
            # preserved), so the gate is a tiny gpsimd copy READING the
            # trigger tile and WRITING into the gated DMA's destination —
            # the write-after-write dependency forces the DMA to wait.
            def gate(trig_qc, dst_ap):
                nc.gpsimd.tensor_copy(out=dst_ap,
                                      in_=x_sb[0:1, trig_qc, 7, 508:512])

            gate(0, wv_sb[0:1, 0, 0:4])
            nc.gpsimd.dma_start(out=wv_sb, in_=wvP[:, :, :])
            gate(0, bv_sb[0:1, 0:4])
            nc.gpsimd.dma_start(out=bv_sb, in_=bv[:, :])
            gate(0, x_sb[0:1, 1, 0, 0:4])
            nc.gpsimd.dma_start(out=x_sb[:, 1], in_=xP[1][:, :, :])
            gate(1, x_sb[0:1, 2, 0, 0:4])
            nc.gpsimd.dma_start(out=x_sb[:, 2], in_=xP[2][:, :, :])
            gate(1, wk_sb[0:1, 1, 0, 0:4])
            nc.gpsimd.dma_start(out=wk_sb[:, 1], in_=wkP[1][:, :, :])
            gate(2, x_sb[0:1, 3, 0, 0:4])
            nc.gpsimd.dma_start(out=x_sb[:, 3], in_=xP[3][:, :, :])
            gate(2, wq_sb[0:1, 1, 0, 0:4])
            nc.gpsimd.dma_start(out=wq_sb[:, 1], in_=wqP[1][:, :, :])

            # ---- static SBUF prep ----
            # Pair-0 prep runs on the DVE now (it finishes before the
            # K00/Q00 bias adds that gate the first scores are ready);
            # the pair-1 qTz zero panels are deferred into block 2 where
            # the DVE is mostly idle.  GpSimd must stay clear for the DMA
            # cascade, and its memsets are ~1.3x slower anyway.
            nc.vector.memset(zt, 0.0)
            # qTz zero halves are written once for the full S up front
            # (variant 0: head-b rows zero; variant 1: head-a rows zero)
            qTz = singles.tile([128, 2, 2, S], BF16)
            kT = singles.tile([128, 2, S], BF16)
            nc.vector.memset(qTz[64:128, 0, 0, :], 0.0)
            nc.vector.memset(qTz[0:64, 0, 1, :], 0.0)
            # v_aug: [128, key_chunk, head, 128]; cols 64:128 are ones, so
            # the ctx matmul emits the softmax denominator into psum
            # partitions 64:128 at no extra cost (matmul cost is N-bound)
            v_aug = singles.tile([128, MC, HPC, 128], BF16)
            nc.vector.memset(v_aug[:, :, :, 64:128], 1.0)

            def qTz_p1_zeros():
                nc.vector.memset(qTz[64:128, 1, 0, :], 0.0)
                nc.vector.memset(qTz[0:64, 1, 1, :], 0.0)

            # ---- projection helpers ----
            def project_T_qc(dst, w_sb, b_sb, p, qc, zpad=False):
                ps = psum.tile([128, 512], F32, tag="proj_ps", name=f"pt_{nc.next_id()}")
                for kc in range(KC):
                    nc.tensor.matmul(
                        ps,
                        lhsT=w_sb[:, p, kc, :],
                        rhs=x_sb[:, qc, kc, :],
                        start=(kc == 0), stop=(kc == KC - 1),
                    )
                qsl = slice(qc * 512, (qc + 1) * 512)
                if zpad:
                    nc.vector.tensor_tensor(
                        out=dst[0:64, p, 0, qsl],
                        in0=ps[0:64, :],
                        in1=b_sb[0:64, p:p + 1].to_broadcast([64, 512]),
                        op=mybir.AluOpType.add,
                    )
                    nc.vector.tensor_tensor(
                        out=dst[64:128, p, 1, qsl],
                        in0=ps[64:128, :],
                        in1=b_sb[64:128, p:p + 1].to_broadcast([64, 512]),
                        op=mybir.AluOpType.add,
                    )
                else:
                    nc.vector.tensor_tensor(
                        out=dst[:, p, qsl],
                        in0=ps,
                        in1=b_sb[:, p:p + 1].to_broadcast([128, 512]),
                        op=mybir.AluOpType.add,
                    )

            def project_v_chunk(mc):
                ps = psum.tile([128, COLS], F32, tag="proj_ps", name=f"pv_{nc.next_id()}")
                for kc in range(KC):
                    nc.tensor.matmul(
                        ps,
                        lhsT=x_sb[:, mc // 4, kc, (mc % 4) * 128:(mc % 4 + 1) * 128],
                        rhs=wv_sb[:, kc, :],
                        start=(kc == 0), stop=(kc == KC - 1),
                    )
                nc.vector.tensor_tensor(
                    out=v_aug[:, mc, :, 0:64],
                    in0=ps[:, :].rearrange("p (h d) -> p h d", h=HPC),
                    in1=bv_sb.rearrange("p (h d) -> p h d", h=HPC),
                    op=mybir.AluOpType.add,
                )
                if apply_mask:
                    nc.vector.tensor_tensor(
                        out=v_aug[:, mc, :, :],
                        in0=v_aug[:, mc, :, :],
                        in1=mm_sb[:, mc:mc + 1, None].to_broadcast([128, HPC, 128]),
                        op=mybir.AluOpType.mult,
                    )

            # Named projection groups: K{p}{j} = kT pair p keys j*512..,
            # Q{p}{j} = qTz pair p queries j*512.. (zero-padded variants).
            def K(p, j):
                return lambda: project_T_qc(kT, wk_sb, bk_sb, p, j)

            def Q(p, j):
                return lambda: project_T_qc(qTz, wq_sb, bq_sb, p, j, zpad=True)

            def V2(mc):
                return lambda: (project_v_chunk(mc), project_v_chunk(mc + 1))

            def emit_ctx(pend):
                p, qc, kc2, e_a, e_b, ctx_a, ctx_b = pend
                ha, hb = 2 * p, 2 * p + 1
                for i, kc in enumerate((2 * kc2, 2 * kc2 + 1)):
                    csl = slice(i * 512, (i + 1) * 512)
                    nc.tensor.matmul(
                        ctx_a, lhsT=v_aug[:, kc, ha, :], rhs=e_a[:, csl],
                        start=(kc == 0), stop=(kc == MC - 1))
                    nc.tensor.matmul(
                        ctx_b, lhsT=v_aug[:, kc, hb, :], rhs=e_b[:, csl],
                        start=(kc == 0), stop=(kc == MC - 1))

            def emit_epilogue(p, qc, ctx_a, ctx_b, last=False):
                # denominator (psum rows 64:128) -> sbuf base-0 tile for the
                # custom-DVE reciprocal; multiply reads ctx psum directly so
                # there is no staging copy of the whole tile.  In the last
                # block the out-DMA issues split across the SP and Scalar
                # sequencers to shorten the tail.
                for h, ctx in ((2 * p, ctx_a), (2 * p + 1, ctx_b)):
                    d0_sb = work.tile([64, 512], F32, tag="den0", name=f"d0_{nc.next_id()}")
                    nc.vector.tensor_copy(out=d0_sb, in_=ctx[64:128, :])
                    d_sb = work.tile([64, 512], F32, tag="den", name=f"d_{nc.next_id()}")
                    nc.vector.reciprocal_approx_fast(out=d_sb, in_=d0_sb)
                    o_sb = work.tile([64, 512], F32, tag="outt", name=f"o_{nc.next_id()}")
                    nc.vector.tensor_tensor(
                        out=o_sb, in0=ctx[0:64, :],
                        in1=d_sb,
                        op=mybir.AluOpType.mult)
                    for piece in range(2):
                        eng = (nc.scalar if piece else nc.sync) if last else nc.sync
                        eng.dma_start(
                            out=out_ext[h][:, qc * 512 + piece * 256:
                                           qc * 512 + (piece + 1) * 256],
                            in_=o_sb[:, piece * 256:(piece + 1) * 256])

            # ---- block schedule ----
            # B1 must own kT0 (all keys), qT0 qc0, every v chunk, and the
            # Q01 prefetch; the other 10 groups go one or two per block so
            # blocks 6-8 (scalar-bound: 16 exps = 16.6us vs 13.7us of
            # attention matmuls) still keep the PE fed.
            blocks = [(0, 0), (0, 1), (0, 2), (0, 3),
                      (1, 0), (1, 1), (1, 2), (1, 3)]
            hooks = [
                # V pair 2j is consumed by ctx(j), which the software
                # pipeline emits during iteration j+1 — so V pairs sit in
                # hook j+1, AFTER that iteration's scores, keeping block
                # 1's first scores/exps ahead of any wv-DMA stall.
                {1: lambda: (K(0, 1)(), V2(0)()),
                 2: V2(2), 3: lambda: (K(0, 2)(), V2(4)()),
                 4: V2(6), 5: lambda: (K(0, 3)(), V2(8)()),
                 6: V2(10), 7: lambda: (Q(0, 1)(), V2(12)(), V2(14)())},
                {1: Q(0, 2), 3: qTz_p1_zeros, 5: K(1, 0)},
                {1: Q(0, 3), 5: K(1, 1)},
                {1: Q(1, 0), 5: K(1, 2)},
                {1: Q(1, 1), 5: K(1, 3)},
                {1: Q(1, 2)},
                {1: Q(1, 3)},
                {},
            ]

            # ---- warmup + first projections ----
            warm_ps = psum.tile([128, 512], F32, tag="proj_ps", name="warm_ps")
            k00_ps = psum.tile([128, 512], F32, tag="proj_ps", name="k00_ps")

            def warm(n, cols=512):
                # singleton-group filler matmuls (start+stop, no accumulate)
                for _ in range(n):
                    nc.tensor.matmul(warm_ps[:, 0:cols], lhsT=zt[:, 0:128],
                                     rhs=zt[:, 0:cols], start=True, stop=True)

            warm(N_WARM)
            # K00 may still be paced by the x qc0 / wk arrivals; short
            # N=128 fillers between steps keep the PE busy so the clock
            # ramp (needs ~3us continuous) isn't reset by the waits.
            for kc in range(KC):
                nc.tensor.matmul(
                    k00_ps, lhsT=wk_sb[:, 0, kc, :],
                    rhs=x_sb[:, 0, kc, :],
                    start=(kc == 0), stop=(kc == KC - 1))
                if kc < KC - 1:
                    warm(3, cols=128)
            nc.vector.tensor_tensor(
                out=kT[:, 0, 0:512], in0=k00_ps,
                in1=bk_sb[:, 0:1].to_broadcast([128, 512]),
                op=mybir.AluOpType.add)
            project_T_qc(qTz, wq_sb, bq_sb, 0, 0, True)   # Q00

            # ---- software-pipelined attention ----
            pend = None
            for bi, (p, qc) in enumerate(blocks):
                qsl = slice(qc * 512, (qc + 1) * 512)
                ctx_a = psum.tile([128, 512], F32, tag="ctx", name=f"ca_{nc.next_id()}")
                ctx_b = psum.tile([128, 512], F32, tag="ctx", name=f"cb_{nc.next_id()}")
                for kc2 in range(MC // 2):
                    kc0, kc1 = 2 * kc2, 2 * kc2 + 1
                    s_a = psum.tile([128, 1024], F32, tag="sps", name=f"sa_{nc.next_id()}")
                    s_b = psum.tile([128, 1024], F32, tag="sps", name=f"sb_{nc.next_id()}")
                    # paired row-group matmuls: head a on array rows 0:63,
                    # head b on rows 64:127 run concurrently
                    for i, kc in enumerate((kc0, kc1)):
                        ksl = slice(kc * 128, (kc + 1) * 128)
                        csl = slice(i * 512, (i + 1) * 512)
                        nc.tensor.matmul(
                            s_a[:, csl], lhsT=kT[:, p, ksl], rhs=qTz[:, p, 0, qsl],
                            start=True, stop=True)
                        nc.tensor.matmul(
                            s_b[:, csl], lhsT=kT[:, p, ksl], rhs=qTz[:, p, 1, qsl],
                            start=True, stop=True)
                    hook = hooks[bi].get(kc2)
                    if hook is not None:
                        hook()
                    e_a = work.tile([128, 1024], BF16, tag="expT", name=f"ea_{nc.next_id()}")
                    e_b = work.tile([128, 1024], BF16, tag="expT", name=f"eb_{nc.next_id()}")
                    nc.scalar.activation(e_a, s_a, mybir.ActivationFunctionType.Exp,
                                         scale=0.125)
                    nc.scalar.activation(e_b, s_b, mybir.ActivationFunctionType.Exp,
                                         scale=0.125)
                    if pend is not None:
                        emit_ctx(pend)
                        if pend[2] == MC // 2 - 1:
                            emit_epilogue(pend[0], pend[1], pend[5], pend[6])
                    pend = (p, qc, kc2, e_a, e_b, ctx_a, ctx_b)
            emit_ctx(pend)
            emit_epilogue(pend[0], pend[1], pend[5], pend[6], last=True)

    nc.compile()
    return nc


def _get_nc(apply_mask: bool) -> bass.Bass:
    if apply_mask not in _CACHE:
        _CACHE[apply_mask] = build(apply_mask)
    return _CACHE[apply_mask]


def _in_maps(x, mask, Wq, bq, Wk, bk, Wv, bv, apply_mask):
    # per-partition-contiguous packing (see build()):
    #   xP[qc, p, kc, c]  = x[b][qc*512+c, kc*128+p]
    #   wP[pair, p, kc, c] = W[kc*128+p, pair*128+c]   (per-core col slice)
    #   wvP[p, kc, c]      = Wv[kc*128+p, c]
    xP_b = [np.ascontiguousarray(
        x[b].reshape(QC, 512, KC, 128).transpose(0, 3, 2, 1)).astype(np_bf16)
        for b in range(B)]

    def pack_w(W, cs):
        Wc = np.asarray(W[:, cs])  # [HID, COLS]
        return np.ascontiguousarray(
            Wc.reshape(KC, 128, 2, 128).transpose(2, 1, 0, 3)).astype(np_bf16)

    maps = []
    for c in range(NCORES):
        b, hg = c // 4, c % 4
        cs = slice(hg * COLS, (hg + 1) * COLS)
        m = {
            "xP": xP_b[b],
            "wqP": pack_w(Wq, cs),
            "wkP": pack_w(Wk, cs),
            "wvP": np.ascontiguousarray(
                Wv[:, cs].reshape(KC, 128, COLS).transpose(1, 0, 2)).astype(np_bf16),
            "bq": np.ascontiguousarray(bq[cs].reshape(2, 128).T).astype(np.float32),
            "bk": np.ascontiguousarray(bk[cs].reshape(2, 128).T).astype(np.float32),
            "bv": np.ascontiguousarray(
                np.broadcast_to(bv[cs], (128, COLS))).astype(np.float32),
        }
        if apply_mask:
            m["maskm"] = np.ascontiguousarray(
                mask[b].astype(np.float32).reshape(MC, 128).T)
        maps.append(m)
    return maps


def _ensure_ntff_hook():
    """The agent image's antenv lacks axon_hooks; synthesize it so
    run_bass_kernel_spmd(trace=True) can reach the axon NTFF profiler."""
    import sys as _sys
    import types as _types
    try:
        from antenv import axon_hooks  # noqa: F401
        return
    except ImportError:
        pass
    import antenv
    mod = _types.ModuleType("antenv.axon_hooks")
    _hook = [None]
    mod.set_axon_ntff_profile_hook = lambda h: _hook.__setitem__(0, h)
    mod.get_axon_ntff_profile_hook = lambda: _hook[0]
    _sys.modules["antenv.axon_hooks"] = mod
    antenv.axon_hooks = mod
    from trn_agent_boot.trn_boot import _ntff_profile_via_ctypes
    mod.set_axon_ntff_profile_hook(
        _ntff_profile_via_ctypes("/opt/axon/libaxon_pjrt.so"))


def run(inputs: dict, trace: bool = False):
    if trace:
        _ensure_ntff_hook()
    x = np.asarray(inputs["x"], dtype=np.float32)
    mask = np.asarray(inputs["mask"])
    apply_mask = not bool((mask == 1).all())
    nc = _get_nc(apply_mask)
    maps = _in_maps(x, mask, np.asarray(inputs["Wq"], np.float32),
                    np.asarray(inputs["bq"], np.float32),
                    np.asarray(inputs["Wk"], np.float32),
                    np.asarray(inputs["bk"], np.float32),
                    np.asarray(inputs["Wv"], np.float32),
                    np.asarray(inputs["bv"], np.float32), apply_mask)
    res = run_bass_kernel_spmd(nc, maps, core_ids=list(range(NCORES)), trace=trace)
    out = np.empty((B, S, HID), dtype=np.float32)
    for c in range(NCORES):
        b, hg = c // 4, c % 4
        cs = slice(hg * COLS, (hg + 1) * COLS)
        ctxT = res.results[c]["out"]          # [HPC, D, S]
        out[b, :, cs] = ctxT.transpose(2, 0, 1).reshape(S, COLS)
    return out, res


def kernel(**inputs) -> np.ndarray:
    out, _ = run(inputs)
    return out


# revision 29
# speedup vs baseline: 1.2328x; 1.0026x over previous
"""Multi-head attention (B=2, S=2048, H=16, D=64) on 8 TRN2 NeuronCores.

Sharding: data parallel on batch (2) x tensor parallel on heads (16 -> 4 per
core).  Core c handles batch c//4 and heads [4*(c%4), 4*(c%4)+4).  Each core
projects q/k/v for its head group from its batch's activations, runs the
full S x S attention for its 4 heads, and writes ctx in [head, D, S] layout.
The host transposes/concatenates shards (not part of HW exec time).

Device kernel (per core, identical SPMD program, no collectives):
  - qT/kT computed directly in [D, S] layout (head pairs packed into 128
    partitions) so the scores matmul needs no transposes.
  - scoresT tiles [S_k=128, S_q] = kT_chunk.T @ qTz; softmax denominator via
    ones columns appended to v (one matmul stream produces ctx and denom).
  - exp on the scalar engine with the 1/sqrt(D) scale folded in; bf16
    matmul operands, f32 PSUM accumulation; final normalize = DVE divide.
  - padding mask folded into v_aug row zeroing (exp(x-1e4) underflows to 0
    in f32, so zeroing masked key rows is exactly equivalent).

Scheduling (see the block schedule below):
  - 8 blocks B(p,qc); block 1 carries only its mandatory projections
    (kT0 all keys, qT0 qc0, all v chunks, plus Q01 prefetch); the
    remaining 10 projection groups are spread one or two per block so the
    scalar-engine-bound pair-1 phase keeps the PE busy too.
  - ctx matmuls lag one (block, kc2) step behind the score matmuls
    (software pipelining across block boundaries) so the PE computes the
    next scores while the scalar engine is on the previous exps.
  - DMA: host packs every tensor per-partition-contiguous so each piece is
    one dma_start with 128 large descriptors.  The DMA fabric round-robins
    all in-flight transfers at ~300GB/s aggregate, so arrival priority =
    not being in flight with anything else: later pieces are gated on
    earlier completions via tiny gpsimd reads (cascade), and only the
    block-1-critical pieces (x qc0, pair-0 weights) issue immediately.
  - memset-fed warmup matmuls bridge the DMA head so the PE never idles
    long enough for the clock governor to hold it at 1.2GHz.
  - epilogue per head: copy denominator psum->sbuf, fast reciprocal,
    multiply straight out of psum (no [128,512] staging copy).
"""

import numpy as np
import ml_dtypes

import concourse.bass as bass
import concourse.tile as tile
from concourse import bacc, mybir
from concourse.bass_utils import run_bass_kernel_spmd

B, S, H, D = 2, 2048, 16, 64
HID = H * D
NCORES = 8
HPC = 4               # heads per core
COLS = HPC * D        # 256 projection columns per core
KC = HID // 128       # 8 contraction chunks for projections
QC = S // 512         # 4 query chunks of 512
MC = S // 128         # 16 key chunks of 128

BF16 = mybir.dt.bfloat16
F32 = mybir.dt.float32
np_bf16 = ml_dtypes.bfloat16

N_WARM = 8            # memset-fed filler matmuls bridging the DMA head

_CACHE = {}


def build(apply_mask: bool) -> bass.Bass:
    nc = bacc.Bacc(None, target_bir_lowering=False, debug=False)

    # Host-packed, per-partition-contiguous layouts (one big descriptor per
    # partition per piece): xP[qc][p][kc][c], w{q,k}P[pair][p][kc][c],
    # wvP[p][kc][c].
    xP = nc.declare_dram_parameter("xP", [QC, 128, KC, 512], BF16, isOutput=False)
    wqP = nc.declare_dram_parameter("wqP", [2, 128, KC, 128], BF16, isOutput=False)
    wkP = nc.declare_dram_parameter("wkP", [2, 128, KC, 128], BF16, isOutput=False)
    wvP = nc.declare_dram_parameter("wvP", [128, KC, COLS], BF16, isOutput=False)
    bq = nc.declare_dram_parameter("bq", [128, 2], F32, isOutput=False)
    bk = nc.declare_dram_parameter("bk", [128, 2], F32, isOutput=False)
    bv = nc.declare_dram_parameter("bv", [128, COLS], F32, isOutput=False)
    if apply_mask:
        mm_in = nc.declare_dram_parameter("maskm", [128, MC], F32, isOutput=False)
    out_ext = nc.declare_dram_parameter("out", [HPC, D, S], F32, isOutput=True)

    with tile.TileContext(nc) as tc:
        with (
            tc.tile_pool(name="singles", bufs=1) as singles,
            tc.tile_pool(name="work", bufs=4) as work,
            tc.tile_pool(name="psum", bufs=2, space="PSUM") as psum,
        ):
            wq_sb = singles.tile([128, 2, KC, 128], BF16)
            wk_sb = singles.tile([128, 2, KC, 128], BF16)
            wv_sb = singles.tile([128, KC, COLS], BF16)
            x_sb = singles.tile([128, QC, KC, 512], BF16)
            bq_sb = singles.tile([128, 2], F32)
            bk_sb = singles.tile([128, 2], F32)
            bv_sb = singles.tile([128, COLS], F32)
            if apply_mask:
                mm_sb = singles.tile([128, MC], F32)
            zt = singles.tile([128, 512], BF16)

            # ---- input DMA ----
            # Immediate wave: only what block 1 needs first (x qc0 + pair-0
            # weights + biases); ~1.5MB in flight -> all landed by ~14us.
            nc.sync.dma_start(out=x_sb[:, 0], in_=xP[0][:, :, :])
            nc.scalar.dma_start(out=wk_sb[:, 0], in_=wkP[0][:, :, :])
            nc.scalar.dma_start(out=wq_sb[:, 0], in_=wqP[0][:, :, :])
            nc.gpsimd.dma_start(out=bk_sb, in_=bk[:, :])
            nc.gpsimd.dma_start(out=bq_sb, in_=bq[:, :])
            if apply_mask:
                nc.gpsimd.dma_start(out=mm_sb, in_=mm_in[:, :])

            # Cascade: each later piece may only enter the shared DMA pipe
            # after an earlier piece has fully landed.  The engine
            # scheduler reorders by data deps (program order is not
            # preserved), so the gate is a tiny gpsimd copy READING the
            # trigger tile and WRITING into the gated DMA's destination —
            # the write-after-write dependency forces the DMA to wait.
            def gate(trig_qc, dst_ap):
                nc.gpsimd.tensor_copy(out=dst_ap,
                                      in_=x_sb[0:1, trig_qc, 7, 508:512])

            def gate_wv(dst_ap):
                nc.gpsimd.tensor_copy(out=dst_ap, in_=wv_sb[0:1, 7, 252:256])

            # x qc1 first and alone (K01 in block 1 needs it by ~19.5us),
            # then wv/bv (ctx(0) needs v_aug by ~20.5), then the rest.
            gate(0, x_sb[0:1, 1, 0, 0:4])
            nc.gpsimd.dma_start(out=x_sb[:, 1], in_=xP[1][:, :, :])
            gate(1, wv_sb[0:1, 0, 0:4])
            nc.gpsimd.dma_start(out=wv_sb, in_=wvP[:, :, :])
            gate(1, bv_sb[0:1, 0:4])
            nc.gpsimd.dma_start(out=bv_sb, in_=bv[:, :])
            gate_wv(x_sb[0:1, 2, 0, 0:4])
            nc.gpsimd.dma_start(out=x_sb[:, 2], in_=xP[2][:, :, :])
            gate(2, x_sb[0:1, 3, 0, 0:4])
            nc.gpsimd.dma_start(out=x_sb[:, 3], in_=xP[3][:, :, :])
            gate(2, wk_sb[0:1, 1, 0, 0:4])
            nc.gpsimd.dma_start(out=wk_sb[:, 1], in_=wkP[1][:, :, :])
            gate(2, wq_sb[0:1, 1, 0, 0:4])
            nc.gpsimd.dma_start(out=wq_sb[:, 1], in_=wqP[1][:, :, :])

            # ---- static SBUF prep ----
            # Pair-0 prep runs on the DVE now (it finishes before the
            # K00/Q00 bias adds that gate the first scores are ready);
            # the pair-1 qTz zero panels are deferred into block 2 where
            # the DVE is mostly idle.  GpSimd must stay clear for the DMA
            # cascade, and its memsets are ~1.3x slower anyway.
            nc.vector.memset(zt, 0.0)
            # qTz zero halves are written once for the full S up front
            # (variant 0: head-b rows zero; variant 1: head-a rows zero)
            qTz = singles.tile([128, 2, 2, S], BF16)
            kT = singles.tile([128, 2, S], BF16)
            nc.vector.memset(qTz[64:128, 0, 0, :], 0.0)
            nc.vector.memset(qTz[0:64, 0, 1, :], 0.0)
            # v_aug: [128, key_chunk, head, 128]; cols 64:128 are ones, so
            # the ctx matmul emits the softmax denominator into psum
            # partitions 64:128 at no extra cost (matmul cost is N-bound)
            v_aug = singles.tile([128, MC, HPC, 128], BF16)
            nc.vector.memset(v_aug[:, :, :, 64:128], 1.0)

            def qTz_p1_zeros():
                nc.vector.memset(qTz[64:128, 1, 0, :], 0.0)
                nc.vector.memset(qTz[0:64, 1, 1, :], 0.0)

            # ---- projection helpers ----
            def project_T_qc(dst, w_sb, b_sb, p, qc, zpad=False):
                ps = psum.tile([128, 512], F32, tag="proj_ps", name=f"pt_{nc.next_id()}")
                for kc in range(KC):
                    nc.tensor.matmul(
                        ps,
                        lhsT=w_sb[:, p, kc, :],
                        rhs=x_sb[:, qc, kc, :],
                        start=(kc == 0), stop=(kc == KC - 1),
                    )
                qsl = slice(qc * 512, (qc + 1) * 512)
                if zpad:
                    nc.vector.tensor_tensor(
                        out=dst[0:64, p, 0, qsl],
                        in0=ps[0:64, :],
                        in1=b_sb[0:64, p:p + 1].to_broadcast([64, 512]),
                        op=mybir.AluOpType.add,
                    )
                    nc.vector.tensor_tensor(
                        out=dst[64:128, p, 1, qsl],
                        in0=ps[64:128, :],
                        in1=b_sb[64:128, p:p + 1].to_broadcast([64, 512]),
                        op=mybir.AluOpType.add,
                    )
                else:
                    nc.vector.tensor_tensor(
                        out=dst[:, p, qsl],
                        in0=ps,
                        in1=b_sb[:, p:p + 1].to_broadcast([128, 512]),
                        op=mybir.AluOpType.add,
                    )

            def project_v_chunk(mc):
                ps = psum.tile([128, COLS], F32, tag="proj_ps", name=f"pv_{nc.next_id()}")
                for kc in range(KC):
                    nc.tensor.matmul(
                        ps,
                        lhsT=x_sb[:, mc // 4, kc, (mc % 4) * 128:(mc % 4 + 1) * 128],
                        rhs=wv_sb[:, kc, :],
                        start=(kc == 0), stop=(kc == KC - 1),
                    )
                nc.vector.tensor_tensor(
                    out=v_aug[:, mc, :, 0:64],
                    in0=ps[:, :].rearrange("p (h d) -> p h d", h=HPC),
                    in1=bv_sb.rearrange("p (h d) -> p h d", h=HPC),
                    op=mybir.AluOpType.add,
                )
                if apply_mask:
                    nc.vector.tensor_tensor(
                        out=v_aug[:, mc, :, :],
                        in0=v_aug[:, mc, :, :],
                        in1=mm_sb[:, mc:mc + 1, None].to_broadcast([128, HPC, 128]),
                        op=mybir.AluOpType.mult,
                    )

            # Named projection groups: K{p}{j} = kT pair p keys j*512..,
            # Q{p}{j} = qTz pair p queries j*512.. (zero-padded variants).
            def K(p, j):
                return lambda: project_T_qc(kT, wk_sb, bk_sb, p, j)

            def Q(p, j):
                return lambda: project_T_qc(qTz, wq_sb, bq_sb, p, j, zpad=True)

            def V2(mc):
                return lambda: (project_v_chunk(mc), project_v_chunk(mc + 1))

            def emit_ctx(pend):
                p, qc, kc2, e_a, e_b, ctx_a, ctx_b = pend
                ha, hb = 2 * p, 2 * p + 1
                for i, kc in enumerate((2 * kc2, 2 * kc2 + 1)):
                    csl = slice(i * 512, (i + 1) * 512)
                    nc.tensor.matmul(
                        ctx_a, lhsT=v_aug[:, kc, ha, :], rhs=e_a[:, csl],
                        start=(kc == 0), stop=(kc == MC - 1))
                    nc.tensor.matmul(
                        ctx_b, lhsT=v_aug[:, kc, hb, :], rhs=e_b[:, csl],
                        start=(kc == 0), stop=(kc == MC - 1))

            def emit_epilogue(p, qc, ctx_a, ctx_b, last=False):
                # denominator (psum rows 64:128) -> sbuf base-0 tile for the
                # custom-DVE reciprocal; multiply reads ctx psum directly so
                # there is no staging copy of the whole tile.  In the last
                # block the out-DMA issues split across the SP and Scalar
                # sequencers to shorten the tail.
                for h, ctx in ((2 * p, ctx_a), (2 * p + 1, ctx_b)):
                    d0_sb = work.tile([64, 512], F32, tag="den0", name=f"d0_{nc.next_id()}")
                    nc.vector.tensor_copy(out=d0_sb, in_=ctx[64:128, :])
                    d_sb = work.tile([64, 512], F32, tag="den", name=f"d_{nc.next_id()}")
                    nc.vector.reciprocal_approx_fast(out=d_sb, in_=d0_sb)
                    o_sb = work.tile([64, 512], F32, tag="outt", name=f"o_{nc.next_id()}")
                    nc.vector.tensor_tensor(
                        out=o_sb, in0=ctx[0:64, :],
                        in1=d_sb,
                        op=mybir.AluOpType.mult)
                    for piece in range(2):
                        eng = (nc.scalar if piece else nc.sync) if last else nc.sync
                        eng.dma_start(
                            out=out_ext[h][:, qc * 512 + piece * 256:
                                           qc * 512 + (piece + 1) * 256],
                            in_=o_sb[:, piece * 256:(piece + 1) * 256])

            # ---- block schedule ----
            # B1 must own kT0 (all keys), qT0 qc0, every v chunk, and the
            # Q01 prefetch; the other 10 groups go one or two per block so
            # blocks 6-8 (scalar-bound: 16 exps = 16.6us vs 13.7us of
            # attention matmuls) still keep the PE fed.
            blocks = [(0, 0), (0, 1), (0, 2), (0, 3),
                      (1, 0), (1, 1), (1, 2), (1, 3)]
            hooks = [
                # V pair 2j is consumed by ctx(j), which the software
                # pipeline emits during iteration j+1 — so V pairs sit in
                # hook j+1, AFTER that iteration's scores, keeping block
                # 1's first scores/exps ahead of any wv-DMA stall.
                {1: lambda: (K(0, 1)(), V2(0)()),
                 2: V2(2), 3: lambda: (K(0, 2)(), V2(4)()),
                 4: V2(6), 5: lambda: (K(0, 3)(), V2(8)()),
                 6: V2(10), 7: lambda: (Q(0, 1)(), V2(12)(), V2(14)())},
                {1: Q(0, 2), 3: qTz_p1_zeros, 5: K(1, 0)},
                {1: Q(0, 3), 5: K(1, 1)},
                {1: Q(1, 0), 5: K(1, 2)},
                {1: Q(1, 1), 5: K(1, 3)},
                {1: Q(1, 2)},
                {1: Q(1, 3)},
                {},
            ]

            # ---- warmup + first projections ----
            warm_ps = psum.tile([128, 512], F32, tag="proj_ps", name="warm_ps")
            k00_ps = psum.tile([128, 512], F32, tag="proj_ps", name="k00_ps")

            def warm(n, cols=512):
                # singleton-group filler matmuls (start+stop, no accumulate)
                for _ in range(n):
                    nc.tensor.matmul(warm_ps[:, 0:cols], lhsT=zt[:, 0:128],
                                     rhs=zt[:, 0:cols], start=True, stop=True)

            warm(N_WARM)
            # K00 may still be paced by the x qc0 / wk arrivals; short
            # N=128 fillers between steps keep the PE busy so the clock
            # ramp (needs ~3us continuous) isn't reset by the waits.
            for kc in range(KC):
                nc.tensor.matmul(
                    k00_ps, lhsT=wk_sb[:, 0, kc, :],
                    rhs=x_sb[:, 0, kc, :],
                    start=(kc == 0), stop=(kc == KC - 1))
                if kc < KC - 1:
                    warm(3, cols=128)
            nc.vector.tensor_tensor(
                out=kT[:, 0, 0:512], in0=k00_ps,
                in1=bk_sb[:, 0:1].to_broadcast([128, 512]),
                op=mybir.AluOpType.add)
            project_T_qc(qTz, wq_sb, bq_sb, 0, 0, True)   # Q00

            # ---- software-pipelined attention ----
            pend = None
            for bi, (p, qc) in enumerate(blocks):
                qsl = slice(qc * 512, (qc + 1) * 512)
                ctx_a = psum.tile([128, 512], F32, tag="ctx", name=f"ca_{nc.next_id()}")
                ctx_b = psum.tile([128, 512], F32, tag="ctx", name=f"cb_{nc.next_id()}")
                for kc2 in range(MC // 2):
                    kc0, kc1 = 2 * kc2, 2 * kc2 + 1
                    s_a = psum.tile([128, 1024], F32, tag="sps", name=f"sa_{nc.next_id()}")
                    s_b = psum.tile([128, 1024], F32, tag="sps", name=f"sb_{nc.next_id()}")
                    # paired row-group matmuls: head a on array rows 0:63,
                    # head b on rows 64:127 run concurrently
                    for i, kc in enumerate((kc0, kc1)):
                        ksl = slice(kc * 128, (kc + 1) * 128)
                        csl = slice(i * 512, (i + 1) * 512)
                        nc.tensor.matmul(
                            s_a[:, csl], lhsT=kT[:, p, ksl], rhs=qTz[:, p, 0, qsl],
                            start=True, stop=True)
                        nc.tensor.matmul(
                            s_b[:, csl], lhsT=kT[:, p, ksl], rhs=qTz[:, p, 1, qsl],
                            start=True, stop=True)
                    hook = hooks[bi].get(kc2)
                    if hook is not None:
                        hook()
                    e_a = work.tile([128, 1024], BF16, tag="expT", name=f"ea_{nc.next_id()}")
                    e_b = work.tile([128, 1024], BF16, tag="expT", name=f"eb_{nc.next_id()}")
                    nc.scalar.activation(e_a, s_a, mybir.ActivationFunctionType.Exp,
                                         scale=0.125)
                    nc.scalar.activation(e_b, s_b, mybir.ActivationFunctionType.Exp,
                                         scale=0.125)
                    if pend is not None:
                        emit_ctx(pend)
                        if pend[2] == MC // 2 - 1:
                            emit_epilogue(pend[0], pend[1], pend[5], pend[6])
                    pend = (p, qc, kc2, e_a, e_b, ctx_a, ctx_b)
            emit_ctx(pend)
            emit_epilogue(pend[0], pend[1], pend[5], pend[6], last=True)

    nc.compile()
    return nc


def _get_nc(apply_mask: bool) -> bass.Bass:
    if apply_mask not in _CACHE:
        _CACHE[apply_mask] = build(apply_mask)
    return _CACHE[apply_mask]


def _in_maps(x, mask, Wq, bq, Wk, bk, Wv, bv, apply_mask):
    # per-partition-contiguous packing (see build()):
    #   xP[qc, p, kc, c]  = x[b][qc*512+c, kc*128+p]
    #   wP[pair, p, kc, c] = W[kc*128+p, pair*128+c]   (per-core col slice)
    #   wvP[p, kc, c]      = Wv[kc*128+p, c]
    xP_b = [np.ascontiguousarray(
        x[b].reshape(QC, 512, KC, 128).transpose(0, 3, 2, 1)).astype(np_bf16)
        for b in range(B)]

    def pack_w(W, cs):
        Wc = np.asarray(W[:, cs])  # [HID, COLS]
        return np.ascontiguousarray(
            Wc.reshape(KC, 128, 2, 128).transpose(2, 1, 0, 3)).astype(np_bf16)

    maps = []
    for c in range(NCORES):
        b, hg = c // 4, c % 4
        cs = slice(hg * COLS, (hg + 1) * COLS)
        m = {
            "xP": xP_b[b],
            "wqP": pack_w(Wq, cs),
            "wkP": pack_w(Wk, cs),
            "wvP": np.ascontiguousarray(
                Wv[:, cs].reshape(KC, 128, COLS).transpose(1, 0, 2)).astype(np_bf16),
            "bq": np.ascontiguousarray(bq[cs].reshape(2, 128).T).astype(np.float32),
            "bk": np.ascontiguousarray(bk[cs].reshape(2, 128).T).astype(np.float32),
            "bv": np.ascontiguousarray(
                np.broadcast_to(bv[cs], (128, COLS))).astype(np.float32),
        }
        if apply_mask:
            m["maskm"] = np.ascontiguousarray(
                mask[b].astype(np.float32).reshape(MC, 128).T)
        maps.append(m)
    return maps


def _ensure_ntff_hook():
    """The agent image's antenv lacks axon_hooks; synthesize it so
    run_bass_kernel_spmd(trace=True) can reach the axon NTFF profiler."""
    import sys as _sys
    import types as _types
    try:
        from antenv import axon_hooks  # noqa: F401
        return
    except ImportError:
        pass
    import antenv
    mod = _types.ModuleType("antenv.axon_hooks")
    _hook = [None]
    mod.set_axon_ntff_profile_hook = lambda h: _hook.__setitem__(0, h)
    mod.get_axon_ntff_profile_hook = lambda: _hook[0]
    _sys.modules["antenv.axon_hooks"] = mod
    antenv.axon_hooks = mod
    from trn_agent_boot.trn_boot import _ntff_profile_via_ctypes
    mod.set_axon_ntff_profile_hook(
        _ntff_profile_via_ctypes("/opt/axon/libaxon_pjrt.so"))


def run(inputs: dict, trace: bool = False):
    if trace:
        _ensure_ntff_hook()
    x = np.asarray(inputs["x"], dtype=np.float32)
    mask = np.asarray(inputs["mask"])
    apply_mask = not bool((mask == 1).all())
    nc = _get_nc(apply_mask)
    maps = _in_maps(x, mask, np.asarray(inputs["Wq"], np.float32),
                    np.asarray(inputs["bq"], np.float32),
                    np.asarray(inputs["Wk"], np.float32),
                    np.asarray(inputs["bk"], np.float32),
                    np.asarray(inputs["Wv"], np.float32),
                    np.asarray(inputs["bv"], np.float32), apply_mask)
    res = run_bass_kernel_spmd(nc, maps, core_ids=list(range(NCORES)), trace=trace)
    out = np.empty((B, S, HID), dtype=np.float32)
    for c in range(NCORES):
        b, hg = c // 4, c % 4
        cs = slice(hg * COLS, (hg + 1) * COLS)
        ctxT = res.results[c]["out"]          # [HPC, D, S]
        out[b, :, cs] = ctxT.transpose(2, 0, 1).reshape(S, COLS)
    return out, res


def kernel(**inputs) -> np.ndarray:
    out, _ = run(inputs)
    return out
